# revision 28
# baseline (speedup 1.0000x reference)
"""LightGCN on 8 Trainium2 NeuronCores (Bass/Tile).

Scheme (src-sharded, ReduceScatter):
- Node table padded to N_PAD = 8*18816 rows, stored per core as a bf16
  [WIN, 128] window table (cols 0:64 = features, 64:128 = don't-care pad so
  rows are 256B for dma_gather).
- Edge (src, dst, val) is processed by core src // WIN. Per core, edges are
  bucketed into 1176 dst-blocks of 128 dsts, laid out in 128-slot tiles;
  per-block tile counts are equalized across cores so one program serves all.
- Per tile: dma_gather x[src] rows (bf16), build the one-hot
  S[slot, dstcol] = (scol == iota) with one DVE op, fold val into the
  gathered rows (gv = g * val, DVE), TensorEngine matmul S^T @ gv
  accumulates the block's [128, 64] partial sums in PSUM.
- Gathers round-robin over 4 SWDGE queues (num_swdge_queues=4) into
  independent tiles so the 4 gpsimd core pairs generate descriptors in
  parallel (the single-queue ucode is the kernel's bottleneck at ~9ns/row).
- Partials are cast to bf16 and go to a permuted DRAM table P' so a chunked
  ReduceScatter (7 chunks of 21*128 rows per window) hands each core exactly
  its window's new embeddings. The RS dispatch is emitted 2 gather-calls
  after a chunk completes and the unshard/acc/table-write 12 calls after, so
  their semaphore waits don't stall the in-order gpsimd/vector/scalar/sync
  queues that the gather pipeline runs on.
- idx/scol/val are laid out per-call-contiguous in DRAM ([n_ci*128, .])
  so each call's metadata load is a few large linear DMA packets; scol/val
  load via the scalar-engine HWDGE queue to stay off the sync queue.
- acc accumulates per-shard in SBUF f32; final AllGather + windowed gathers
  of user/item rows + dot products produce gamma.
"""
import os
import numpy as np
import ml_dtypes

BF = ml_dtypes.bfloat16
LAST_RESULTS = None  # BassKernelResults of the most recent run (for test.py)

N_USERS = 100000
N_ITEMS = 50000
N = N_USERS + N_ITEMS
D = 64
NC = 8
WIN = 18816             # 147 * 128 nodes per core window
N_PAD = NC * WIN        # 150528
NBLOCKS = N_PAD // 128  # 1176
NCHUNK = 21
QROWS = WIN // NCHUNK   # 896 = 7*128
CHROWS = NC * QROWS     # 50176 rows per P' chunk
BPC = NBLOCKS // NCHUNK # 392 blocks per chunk
ROWB = 128              # table row elems (256B)
NB = 48                 # tiles per dma_gather call
NBV = 24                # tiles of the val-multiply done on DVE (rest scalar)
N_LAYERS = 3


def _chunk_of_block(b):
    return ((b * 128) % WIN) // QROWS


def _block_pprime_row(b):
    n = b * 128
    w, r = divmod(n, WIN)
    q, k = divmod(r, QROWS)
    return (q * NC + w) * QROWS + k


def _prep_edges(edge_src, edge_dst, edge_val):
    src = edge_src.astype(np.int64)
    dst = edge_dst.astype(np.int64)
    val = edge_val.astype(np.float32)
    core = src // WIN
    blk = dst // 128

    counts = np.zeros((NC, NBLOCKS), dtype=np.int64)
    np.add.at(counts, (core, blk), 1)
    ntiles = np.maximum(1, (counts.max(axis=0) + 127) // 128)

    order = sorted(range(NBLOCKS), key=lambda b: (_chunk_of_block(b), b))
    # pad total tiles to a multiple of NB (extra zero tiles on the last block)
    tot = int(ntiles[order].sum())
    pad_tiles = (-tot) % NB
    ntiles[order[-1]] += pad_tiles
    sched = [(b, int(ntiles[b])) for b in order]

    slot_of_block = {}
    off = 0
    for b, t in sched:
        slot_of_block[b] = off
        off += t * 128
    S = off

    ordk = np.lexsort((dst, blk, core))
    src_s, dst_s, val_s, core_s, blk_s = (
        src[ordk], dst[ordk], val[ordk], core[ordk], blk[ordk])
    key = core_s * NBLOCKS + blk_s
    per_core = []
    for c in range(NC):
        idx16 = np.zeros(S, dtype=np.int16)
        scol = np.zeros(S, dtype=np.float32)
        v = np.zeros(S, dtype=np.float32)
        lo = np.searchsorted(key, c * NBLOCKS)
        hi = np.searchsorted(key, (c + 1) * NBLOCKS)
        bsrc, bdst, bval, bblk = (
            src_s[lo:hi], dst_s[lo:hi], val_s[lo:hi], blk_s[lo:hi])
        bounds = np.searchsorted(bblk, np.arange(NBLOCKS + 1))
        for b in range(NBLOCKS):
            a, e = bounds[b], bounds[b + 1]
            if e == a:
                continue
            o = slot_of_block[b]
            n = e - a
            idx16[o:o + n] = (bsrc[a:e] - c * WIN).astype(np.int16)
            scol[o:o + n] = (bdst[a:e] - b * 128).astype(np.float32)
            v[o:o + n] = bval[a:e]
        per_core.append((idx16, scol, v))
    return sched, per_core, S


def _wrap_idx(idx_flat):
    S = idx_flat.shape[0]
    w = idx_flat.reshape(S // 16, 16).T
    return np.tile(w, (8, 1)).copy()


def _wrap_idx_ci(idx_flat):
    # [n_ci*128, NB*8]: per-ci contiguous blocks of the wrapped image
    w = _wrap_idx(idx_flat)              # [128, T*8]
    T8 = w.shape[1]
    n_ci = T8 // (NB * 8)
    return np.concatenate(
        [w[:, i * NB * 8:(i + 1) * NB * 8] for i in range(n_ci)],
        axis=0).copy()


def _wrap_slots_ci(a_flat):
    w = _wrap_slots(a_flat)              # [128, T]
    T = w.shape[1]
    n_ci = T // NB
    return np.concatenate(
        [w[:, i * NB:(i + 1) * NB] for i in range(n_ci)], axis=0).copy()


def _wrap_slots(a_flat):
    S = a_flat.shape[0]
    return a_flat.reshape(S // 128, 128).T.copy()


def _prep_final(users, items):
    B = users.shape[0]
    pcn = B // NC
    u = users.astype(np.int64)
    it = items.astype(np.int64) + N_USERS
    uw = u // WIN
    iw = it // WIN
    combos = sorted({(int(a), int(b)) for a, b in zip(uw, iw)})
    gmax = {}
    for c in range(NC):
        lo, hi = c * pcn, (c + 1) * pcn
        for cu, ci in combos:
            m = int(((uw[lo:hi] == cu) & (iw[lo:hi] == ci)).sum())
            gmax[(cu, ci)] = max(gmax.get((cu, ci), 0), m)
    combos_sizes = [(k, (gmax[k] + 127) // 128 * 128) for k in combos]
    pad_total = sum(s for _, s in combos_sizes)
    plans = []
    for c in range(NC):
        lo, hi = c * pcn, (c + 1) * pcn
        slots_pair = np.full(pad_total, -1, dtype=np.int64)
        uidx = np.zeros(pad_total, dtype=np.int16)
        iidx = np.zeros(pad_total, dtype=np.int16)
        usub, isub = [], []
        off = 0
        for (cu, ci), size in combos_sizes:
            sel = np.nonzero((uw[lo:hi] == cu) & (iw[lo:hi] == ci))[0]
            n = sel.shape[0]
            slots_pair[off:off + n] = lo + sel
            uidx[off:off + n] = (u[lo + sel] - cu * WIN).astype(np.int16)
            iidx[off:off + n] = (it[lo + sel] - ci * WIN).astype(np.int16)
            usub.append((cu, off, off + size))
            isub.append((ci, off, off + size))
            off += size
        plans.append({"slots_pair": slots_pair, "uidx": uidx, "iidx": iidx,
                      "usub": usub, "isub": isub, "ntot": pad_total})
    return plans, combos_sizes, pad_total


def _build_program(sched, T_tiles, fin_subs, FS):
    """Build + compile the shared 8-core program.

    sched: [(block, ntiles)] chunk-major. T_tiles = total tiles per layer.
    fin_subs: (usub, isub) window sub-ranges (identical across cores).
    FS: final slot count (multiple of 128).
    """
    import concourse.bacc as bacc
    import concourse.tile as tile
    from concourse import mybir

    FT = FS // 128
    nc = bacc.Bacc("TRN2", target_bir_lowering=False, debug=False,
                   num_devices=NC, num_swdge_queues=4)
    dt = mybir.dt

    t0_in = nc.dram_tensor("t0", [WIN, ROWB], dt.bfloat16, kind="ExternalInput")
    x0_in = nc.dram_tensor("x0", [WIN, D], dt.float32, kind="ExternalInput")
    n_ci = T_tiles // NB
    idx_in = nc.dram_tensor("idx", [n_ci * 128, NB * 8], dt.int16,
                            kind="ExternalInput")
    scol_in = nc.dram_tensor("scol", [n_ci * 128, NB], dt.bfloat16,
                             kind="ExternalInput")
    val_in = nc.dram_tensor("val", [n_ci * 128, NB], dt.bfloat16,
                            kind="ExternalInput")
    iota_in = nc.dram_tensor("iota", [128, 128], dt.bfloat16,
                             kind="ExternalInput")
    fu_in = nc.dram_tensor("fuidx", [128, FS // 16], dt.int16,
                           kind="ExternalInput")
    fi_in = nc.dram_tensor("fiidx", [128, FS // 16], dt.int16,
                           kind="ExternalInput")
    gamma_out = nc.dram_tensor("gamma", [128, FT], dt.float32,
                               kind="ExternalOutput")

    tp = [nc.dram_tensor(f"tbl{i}", [WIN, ROWB], dt.bfloat16, kind="Internal")
          for i in range(2)]
    read_t = [t0_in, tp[0], tp[1]]
    write_t = [tp[0], tp[1], None]
    A_tab = nc.dram_tensor("atab", [N_PAD, D], dt.float32, kind="Internal")
    ag_in = nc.dram_tensor("agin", [WIN, D], dt.float32, kind="Internal")
    pp = [[nc.dram_tensor(f"pp_l{l}_q{q}", [CHROWS, D], dt.bfloat16,
                          kind="Internal") for q in range(NCHUNK)]
          for l in range(N_LAYERS)]
    rs = [[nc.dram_tensor(f"rs_l{l}_q{q}", [QROWS, D], dt.bfloat16,
                          kind="Internal")
           for q in range(NCHUNK)] for l in range(N_LAYERS)]

    n_calls = T_tiles // NB
    # flat per-tile (block, is_first, is_last) in sched order
    tiles_meta = []
    for b, nt in sched:
        for k in range(nt):
            tiles_meta.append((b, k == 0, k == nt - 1))
    assert len(tiles_meta) == T_tiles

    with tile.TileContext(nc) as tc:
        with tc.tile_pool(name="persist", bufs=1) as pers, \
             tc.tile_pool(name="gbuf", bufs=4) as gpool, \
             tc.tile_pool(name="sbuf2", bufs=2) as spool, \
             tc.tile_pool(name="rbuf", bufs=2) as rpool, \
             tc.tile_pool(name="psum", bufs=4, space="PSUM") as ppool:

            iota_t = pers.tile([128, 128], dt.bfloat16)
            nc.sync.dma_start(iota_t[:], iota_in[:])
            acc_t = pers.tile([128, NCHUNK, QROWS // 128, D], dt.float32)
            nc.sync.dma_start(
                acc_t[:],
                x0_in.ap().rearrange("(q p t) d -> p q t d",
                                     q=NCHUNK, p=128, t=QROWS // 128))

            qctr = 0            # SWDGE queue round-robin counter

            def dispatch_rs(layer, q):
                nc.gpsimd.collective_compute(
                    "ReduceScatter", mybir.AluOpType.add,
                    replica_groups=[list(range(NC))],
                    ins=[pp[layer][q].ap()],
                    outs=[rs[layer][q].ap()])

            def finish_chunk(layer, q):
                """Unshard + acc for a chunk whose RS has (long since)
                completed. Emitted ~24 calls late so the sem waits on the
                in-order vector/scalar/sync queues are already satisfied."""
                rsb = rpool.tile([128, QROWS // 128, D],
                                 dt.bfloat16, tag="rsb")
                nc.sync.dma_start(
                    rsb[:],
                    rs[layer][q].ap().rearrange(
                        "(p t) d -> p t d", p=128,
                        t=QROWS // 128))
                if write_t[layer] is not None:
                    nc.sync.dma_start(
                        write_t[layer][q * QROWS:(q + 1) * QROWS,
                                       0:D].rearrange(
                            "(p t) d -> p t d", p=128,
                            t=QROWS // 128),
                        rsb[:])
                nc.vector.tensor_add(
                    acc_t[:, q, :, :], acc_t[:, q, :, :], rsb[:])

            pending_rs = []     # [(layer, q, ready_ci)]
            pending_fin = []    # [(layer, q, ready_ci)]
            for layer in range(N_LAYERS):
                tbl = read_t[layer]
                blk_cnt = 0         # completed blocks in current chunk
                psum_t = None
                stage_t = None
                ti = 0
                for ci in range(n_calls):
                    idx_t = gpool.tile([128, NB * 8], dt.int16, tag="idx")
                    nc.sync.dma_start(
                        idx_t[:], idx_in[ci * 128:(ci + 1) * 128, :])
                    scol_t = gpool.tile([128, NB], dt.bfloat16, tag="scol")
                    nc.scalar.dma_start(
                        scol_t[:], scol_in[ci * 128:(ci + 1) * 128, :])
                    val_t = gpool.tile([128, NB], dt.bfloat16, tag="val")
                    nc.scalar.dma_start(
                        val_t[:], val_in[ci * 128:(ci + 1) * 128, :])
                    # ucode limit: 1024 idxs per dma_gather call; separate
                    # tiles so the 4 SWDGE queues actually overlap (slices of
                    # one tile get serialized by the dep tracker)
                    g_ts = []
                    for k in range(NB // 8):
                        g_k = gpool.tile([128, 8, ROWB], dt.bfloat16,
                                         tag=f"g{k}")
                        nc.gpsimd.dma_gather(
                            out_ap=g_k[:],
                            in_ap=tbl[:],
                            idxs_ap=idx_t[:, k * 64:(k + 1) * 64],
                            num_idxs=1024, num_idxs_reg=1024,
                            elem_size=ROWB, queue_num=qctr % 4)
                        qctr += 1
                        g_ts.append(g_k)
                    while pending_rs and pending_rs[0][2] <= ci:
                        lq = pending_rs.pop(0)
                        dispatch_rs(lq[0], lq[1])
                    while pending_fin and pending_fin[0][2] <= ci:
                        lq = pending_fin.pop(0)
                        finish_chunk(lq[0], lq[1])
                    s_t = spool.tile([128, NB, 128], dt.bfloat16, tag="s")
                    nc.vector.tensor_tensor(
                        s_t[:],
                        scol_t[:].unsqueeze(-1).broadcast_to([128, NB, 128]),
                        iota_t[:].unsqueeze(1).broadcast_to([128, NB, 128]),
                        mybir.AluOpType.is_equal)
                    gv_t = spool.tile([128, NB, D], dt.bfloat16, tag="gv")
                    for k in range(NB // 8):
                        nc.vector.tensor_tensor(
                            gv_t[:, k * 8:(k + 1) * 8, :],
                            g_ts[k][:, :, 0:D],
                            val_t[:, k * 8:(k + 1) * 8].unsqueeze(
                                -1).broadcast_to([128, 8, D]),
                            mybir.AluOpType.mult)

                    for t in range(NB):
                        b, first, last = tiles_meta[ti]
                        if first and blk_cnt % 4 == 0:
                            psum_t = ppool.tile([128, 4, D], dt.float32)
                        nc.tensor.matmul(
                            psum_t[:, blk_cnt % 4, :],
                            s_t[:, t, :], gv_t[:, t, :],
                            start=first, stop=last)
                        ti += 1
                        if not last:
                            continue
                        # block b complete
                        if blk_cnt % 8 == 0:
                            stage_t = spool.tile([128, 8, D], dt.bfloat16,
                                                 tag="stage")
                        if blk_cnt % 4 == 3:
                            nc.scalar.activation(
                                stage_t[:, (blk_cnt % 8) - 3:(blk_cnt % 8) + 1, :],
                                psum_t[:],
                                mybir.ActivationFunctionType.Copy)
                        if blk_cnt % 8 == 7:
                            q = _chunk_of_block(b)
                            r0 = _block_pprime_row(b) - q * CHROWS - 7 * 128
                            nc.sync.dma_start(
                                pp[layer][q][r0:r0 + 1024, :].rearrange(
                                    "(j p) d -> p j d", p=128, j=8),
                                stage_t[:])
                        blk_cnt += 1
                        if blk_cnt % BPC == 0:
                            # chunk q done; RS dispatched 2 calls later, the
                            # unshard/acc ~24 calls later (past RS completion)
                            q = blk_cnt // BPC - 1
                            pending_rs.append((layer, q, ci + 2))
                            pending_fin.append((layer, q, ci + 8))
                while pending_rs:
                    lq = pending_rs.pop(0)
                    dispatch_rs(lq[0], lq[1])
                while pending_fin:
                    lq = pending_fin.pop(0)
                    finish_chunk(lq[0], lq[1])

            # final: light = acc/4 -> AllGather -> windowed gathers -> dots
            nc.vector.tensor_scalar_mul(acc_t[:], acc_t[:], 0.25)
            nc.sync.dma_start(
                ag_in.ap().rearrange("(q p t) d -> p q t d",
                                     q=NCHUNK, p=128, t=QROWS // 128),
                acc_t[:])
            nc.gpsimd.collective_compute(
                "AllGather", mybir.AluOpType.bypass,
                replica_groups=[list(range(NC))],
                ins=[ag_in.ap()], outs=[A_tab.ap()])

            fu_t = pers.tile([128, FS // 16], dt.int16)
            nc.sync.dma_start(fu_t[:], fu_in[:])
            fi_t = pers.tile([128, FS // 16], dt.int16)
            nc.sync.dma_start(fi_t[:], fi_in[:])
            ug_t = pers.tile([128, FT, D], dt.float32)
            ig_t = pers.tile([128, FT, D], dt.float32)
            for subs, idxt, outt in ((fin_subs[0], fu_t, ug_t),
                                     (fin_subs[1], fi_t, ig_t)):
                for w, lo, hi in subs:
                    for a in range(lo, hi, 1024):
                        n = min(1024, hi - a)
                        nc.gpsimd.dma_gather(
                            out_ap=outt[:, a // 128:(a + n) // 128, :],
                            in_ap=A_tab[w * WIN:(w + 1) * WIN, :],
                            idxs_ap=idxt[:, a // 16:(a + n) // 16],
                            num_idxs=n, num_idxs_reg=n,
                            elem_size=D, queue_num=qctr % 4)
                        qctr += 1
            prod_t = pers.tile([128, FT, D], dt.float32)
            nc.vector.tensor_mul(prod_t[:], ug_t[:], ig_t[:])
            gam_t = pers.tile([128, FT], dt.float32)
            nc.vector.tensor_reduce(
                gam_t[:].unsqueeze(-1), prod_t[:],
                axis=mybir.AxisListType.X, op=mybir.AluOpType.add)
            nc.sync.dma_start(gamma_out[:], gam_t[:])

    nc.compile()
    return nc


def kernel(**inputs):
    from concourse import bass_utils

    users = np.asarray(inputs["users"])
    items = np.asarray(inputs["items"])
    edge_src = np.asarray(inputs["edge_src"])
    edge_dst = np.asarray(inputs["edge_dst"])
    edge_val = np.asarray(inputs["edge_val"], dtype=np.float32)
    user_emb = np.asarray(inputs["user_emb"], dtype=np.float32)
    item_emb = np.asarray(inputs["item_emb"], dtype=np.float32)

    sched, per_core, S = _prep_edges(edge_src, edge_dst, edge_val)
    T_tiles = S // 128
    plans, combos_sizes, FS = _prep_final(users, items)
    fin_subs = (plans[0]["usub"], plans[0]["isub"])

    nc = _build_program(sched, T_tiles, fin_subs, FS)

    x0 = np.zeros((N_PAD, D), dtype=np.float32)
    x0[:N_USERS] = user_emb
    x0[N_USERS:N] = item_emb
    iota_img = np.tile(np.arange(128, dtype=np.float32).astype(BF)[None, :],
                       (128, 1)).copy()

    in_maps = []
    for c in range(NC):
        idx16, scol, val = per_core[c]
        t0 = np.zeros((WIN, ROWB), dtype=BF)
        t0[:, :D] = x0[c * WIN:(c + 1) * WIN].astype(BF)
        pl = plans[c]
        in_maps.append({
            "t0": t0,
            "x0": x0[c * WIN:(c + 1) * WIN].copy(),
            "idx": _wrap_idx_ci(idx16),
            "scol": _wrap_slots_ci(scol.astype(BF)),
            "val": _wrap_slots_ci(val.astype(BF)),
            "iota": iota_img,
            "fuidx": _wrap_idx(pl["uidx"]),
            "fiidx": _wrap_idx(pl["iidx"]),
        })

    res = bass_utils.run_bass_kernel_spmd(
        nc, in_maps, core_ids=list(range(NC)),
        trace=bool(os.environ.get("KERNEL_TRACE")))
    global LAST_RESULTS
    LAST_RESULTS = res

    gamma = np.zeros(users.shape[0], dtype=np.float32)
    for c in range(NC):
        img = res.results[c]["gamma"]          # [128, FT]
        flat = img.T.reshape(-1)               # slot s = 128*t + p
        pl = plans[c]
        sel = pl["slots_pair"] >= 0
        gamma[pl["slots_pair"][sel]] = flat[sel]
    return gamma



# revision 29
# speedup vs baseline: 1.0048x; 1.0048x over previous
"""LightGCN on 8 Trainium2 NeuronCores (Bass/Tile).

Scheme (src-sharded, ReduceScatter):
- Node table padded to N_PAD = 8*18816 rows, stored per core as a bf16
  [WIN, 128] window table (cols 0:64 = features, 64:128 = don't-care pad so
  rows are 256B for dma_gather).
- Edge (src, dst, val) is processed by core src // WIN. Per core, edges are
  bucketed into 1176 dst-blocks of 128 dsts, laid out in 128-slot tiles;
  per-block tile counts are equalized across cores so one program serves all.
- Per tile: dma_gather x[src] rows (bf16), build the one-hot
  S[slot, dstcol] = (scol == iota) with one DVE op, fold val into the
  gathered rows (gv = g * val, DVE), TensorEngine matmul S^T @ gv
  accumulates the block's [128, 64] partial sums in PSUM.
- Gathers round-robin over 4 SWDGE queues (num_swdge_queues=4) into
  independent tiles so the 4 gpsimd core pairs generate descriptors in
  parallel (the single-queue ucode is the kernel's bottleneck at ~9ns/row).
- Partials are cast to bf16 and go to a permuted DRAM table P' so a chunked
  ReduceScatter (7 chunks of 21*128 rows per window) hands each core exactly
  its window's new embeddings. The RS dispatch is emitted 2 gather-calls
  after a chunk completes and the unshard/acc/table-write 12 calls after, so
  their semaphore waits don't stall the in-order gpsimd/vector/scalar/sync
  queues that the gather pipeline runs on.
- idx/scol/val are laid out per-call-contiguous in DRAM ([n_ci*128, .])
  so each call's metadata load is a few large linear DMA packets; scol/val
  load via the scalar-engine HWDGE queue to stay off the sync queue.
- acc accumulates per-shard in SBUF f32; final AllGather + windowed gathers
  of user/item rows + dot products produce gamma.
"""
import os
import numpy as np
import ml_dtypes

BF = ml_dtypes.bfloat16
LAST_RESULTS = None  # BassKernelResults of the most recent run (for test.py)

N_USERS = 100000
N_ITEMS = 50000
N = N_USERS + N_ITEMS
D = 64
NC = 8
WIN = 18816             # 147 * 128 nodes per core window
N_PAD = NC * WIN        # 150528
NBLOCKS = N_PAD // 128  # 1176
NCHUNK = 7
QROWS = WIN // NCHUNK   # 2688 = 21*128
CHROWS = NC * QROWS     # 50176 rows per P' chunk
BPC = NBLOCKS // NCHUNK # 392 blocks per chunk
ROWB = 128              # table row elems (256B)
NB = 48                 # tiles per dma_gather call
NBV = 24                # tiles of the val-multiply done on DVE (rest scalar)
N_LAYERS = 3


def _chunk_of_block(b):
    return ((b * 128) % WIN) // QROWS


def _block_pprime_row(b):
    n = b * 128
    w, r = divmod(n, WIN)
    q, k = divmod(r, QROWS)
    return (q * NC + w) * QROWS + k


def _prep_edges(edge_src, edge_dst, edge_val):
    src = edge_src.astype(np.int64)
    dst = edge_dst.astype(np.int64)
    val = edge_val.astype(np.float32)
    core = src // WIN
    blk = dst // 128

    counts = np.zeros((NC, NBLOCKS), dtype=np.int64)
    np.add.at(counts, (core, blk), 1)
    ntiles = np.maximum(1, (counts.max(axis=0) + 127) // 128)

    order = sorted(range(NBLOCKS), key=lambda b: (_chunk_of_block(b), b))
    # pad total tiles to a multiple of NB (extra zero tiles on the last block)
    tot = int(ntiles[order].sum())
    pad_tiles = (-tot) % NB
    ntiles[order[-1]] += pad_tiles
    sched = [(b, int(ntiles[b])) for b in order]

    slot_of_block = {}
    off = 0
    for b, t in sched:
        slot_of_block[b] = off
        off += t * 128
    S = off

    ordk = np.lexsort((dst, blk, core))
    src_s, dst_s, val_s, core_s, blk_s = (
        src[ordk], dst[ordk], val[ordk], core[ordk], blk[ordk])
    key = core_s * NBLOCKS + blk_s
    per_core = []
    for c in range(NC):
        idx16 = np.zeros(S, dtype=np.int16)
        scol = np.zeros(S, dtype=np.float32)
        v = np.zeros(S, dtype=np.float32)
        lo = np.searchsorted(key, c * NBLOCKS)
        hi = np.searchsorted(key, (c + 1) * NBLOCKS)
        bsrc, bdst, bval, bblk = (
            src_s[lo:hi], dst_s[lo:hi], val_s[lo:hi], blk_s[lo:hi])
        bounds = np.searchsorted(bblk, np.arange(NBLOCKS + 1))
        for b in range(NBLOCKS):
            a, e = bounds[b], bounds[b + 1]
            if e == a:
                continue
            o = slot_of_block[b]
            n = e - a
            idx16[o:o + n] = (bsrc[a:e] - c * WIN).astype(np.int16)
            scol[o:o + n] = (bdst[a:e] - b * 128).astype(np.float32)
            v[o:o + n] = bval[a:e]
        per_core.append((idx16, scol, v))
    return sched, per_core, S


def _wrap_idx(idx_flat):
    S = idx_flat.shape[0]
    w = idx_flat.reshape(S // 16, 16).T
    return np.tile(w, (8, 1)).copy()


def _wrap_idx_ci(idx_flat):
    # [n_ci*128, NB*8]: per-ci contiguous blocks of the wrapped image
    w = _wrap_idx(idx_flat)              # [128, T*8]
    T8 = w.shape[1]
    n_ci = T8 // (NB * 8)
    return np.concatenate(
        [w[:, i * NB * 8:(i + 1) * NB * 8] for i in range(n_ci)],
        axis=0).copy()


def _wrap_slots_ci(a_flat):
    w = _wrap_slots(a_flat)              # [128, T]
    T = w.shape[1]
    n_ci = T // NB
    return np.concatenate(
        [w[:, i * NB:(i + 1) * NB] for i in range(n_ci)], axis=0).copy()


def _wrap_slots(a_flat):
    S = a_flat.shape[0]
    return a_flat.reshape(S // 128, 128).T.copy()


def _prep_final(users, items):
    B = users.shape[0]
    pcn = B // NC
    u = users.astype(np.int64)
    it = items.astype(np.int64) + N_USERS
    uw = u // WIN
    iw = it // WIN
    combos = sorted({(int(a), int(b)) for a, b in zip(uw, iw)})
    gmax = {}
    for c in range(NC):
        lo, hi = c * pcn, (c + 1) * pcn
        for cu, ci in combos:
            m = int(((uw[lo:hi] == cu) & (iw[lo:hi] == ci)).sum())
            gmax[(cu, ci)] = max(gmax.get((cu, ci), 0), m)
    combos_sizes = [(k, (gmax[k] + 127) // 128 * 128) for k in combos]
    pad_total = sum(s for _, s in combos_sizes)
    plans = []
    for c in range(NC):
        lo, hi = c * pcn, (c + 1) * pcn
        slots_pair = np.full(pad_total, -1, dtype=np.int64)
        uidx = np.zeros(pad_total, dtype=np.int16)
        iidx = np.zeros(pad_total, dtype=np.int16)
        usub, isub = [], []
        off = 0
        for (cu, ci), size in combos_sizes:
            sel = np.nonzero((uw[lo:hi] == cu) & (iw[lo:hi] == ci))[0]
            n = sel.shape[0]
            slots_pair[off:off + n] = lo + sel
            uidx[off:off + n] = (u[lo + sel] - cu * WIN).astype(np.int16)
            iidx[off:off + n] = (it[lo + sel] - ci * WIN).astype(np.int16)
            usub.append((cu, off, off + size))
            isub.append((ci, off, off + size))
            off += size
        plans.append({"slots_pair": slots_pair, "uidx": uidx, "iidx": iidx,
                      "usub": usub, "isub": isub, "ntot": pad_total})
    return plans, combos_sizes, pad_total


def _build_program(sched, T_tiles, fin_subs, FS):
    """Build + compile the shared 8-core program.

    sched: [(block, ntiles)] chunk-major. T_tiles = total tiles per layer.
    fin_subs: (usub, isub) window sub-ranges (identical across cores).
    FS: final slot count (multiple of 128).
    """
    import concourse.bacc as bacc
    import concourse.tile as tile
    from concourse import mybir

    FT = FS // 128
    nc = bacc.Bacc("TRN2", target_bir_lowering=False, debug=False,
                   num_devices=NC, num_swdge_queues=4)
    dt = mybir.dt

    t0_in = nc.dram_tensor("t0", [WIN, ROWB], dt.bfloat16, kind="ExternalInput")
    x0_in = nc.dram_tensor("x0", [WIN, D], dt.float32, kind="ExternalInput")
    n_ci = T_tiles // NB
    idx_in = nc.dram_tensor("idx", [n_ci * 128, NB * 8], dt.int16,
                            kind="ExternalInput")
    scol_in = nc.dram_tensor("scol", [n_ci * 128, NB], dt.bfloat16,
                             kind="ExternalInput")
    val_in = nc.dram_tensor("val", [n_ci * 128, NB], dt.bfloat16,
                            kind="ExternalInput")
    iota_in = nc.dram_tensor("iota", [128, 128], dt.bfloat16,
                             kind="ExternalInput")
    fu_in = nc.dram_tensor("fuidx", [128, FS // 16], dt.int16,
                           kind="ExternalInput")
    fi_in = nc.dram_tensor("fiidx", [128, FS // 16], dt.int16,
                           kind="ExternalInput")
    gamma_out = nc.dram_tensor("gamma", [128, FT], dt.float32,
                               kind="ExternalOutput")

    tp = [nc.dram_tensor(f"tbl{i}", [WIN, ROWB], dt.bfloat16, kind="Internal")
          for i in range(2)]
    read_t = [t0_in, tp[0], tp[1]]
    write_t = [tp[0], tp[1], None]
    A_tab = nc.dram_tensor("atab", [N_PAD, D], dt.float32, kind="Internal")
    ag_in = nc.dram_tensor("agin", [WIN, D], dt.float32, kind="Internal")
    pp = [[nc.dram_tensor(f"pp_l{l}_q{q}", [CHROWS, D], dt.bfloat16,
                          kind="Internal") for q in range(NCHUNK)]
          for l in range(N_LAYERS)]
    rs = [[nc.dram_tensor(f"rs_l{l}_q{q}", [QROWS, D], dt.bfloat16,
                          kind="Internal")
           for q in range(NCHUNK)] for l in range(N_LAYERS)]

    n_calls = T_tiles // NB
    # flat per-tile (block, is_first, is_last) in sched order
    tiles_meta = []
    for b, nt in sched:
        for k in range(nt):
            tiles_meta.append((b, k == 0, k == nt - 1))
    assert len(tiles_meta) == T_tiles

    with tile.TileContext(nc) as tc:
        with tc.tile_pool(name="persist", bufs=1) as pers, \
             tc.tile_pool(name="gbuf", bufs=4) as gpool, \
             tc.tile_pool(name="sbuf2", bufs=2) as spool, \
             tc.tile_pool(name="rbuf", bufs=2) as rpool, \
             tc.tile_pool(name="psum", bufs=4, space="PSUM") as ppool:

            iota_t = pers.tile([128, 128], dt.bfloat16)
            nc.sync.dma_start(iota_t[:], iota_in[:])
            acc_t = pers.tile([128, NCHUNK, QROWS // 128, D], dt.float32)
            nc.sync.dma_start(
                acc_t[:],
                x0_in.ap().rearrange("(q p t) d -> p q t d",
                                     q=NCHUNK, p=128, t=QROWS // 128))

            qctr = 0            # SWDGE queue round-robin counter

            def dispatch_rs(layer, q):
                nc.gpsimd.collective_compute(
                    "ReduceScatter", mybir.AluOpType.add,
                    replica_groups=[list(range(NC))],
                    ins=[pp[layer][q].ap()],
                    outs=[rs[layer][q].ap()])

            def finish_chunk(layer, q):
                """Unshard + acc for a chunk whose RS has (long since)
                completed. Emitted ~24 calls late so the sem waits on the
                in-order vector/scalar/sync queues are already satisfied."""
                rsb = rpool.tile([128, QROWS // 128, D],
                                 dt.bfloat16, tag="rsb")
                nc.sync.dma_start(
                    rsb[:],
                    rs[layer][q].ap().rearrange(
                        "(p t) d -> p t d", p=128,
                        t=QROWS // 128))
                if write_t[layer] is not None:
                    nc.sync.dma_start(
                        write_t[layer][q * QROWS:(q + 1) * QROWS,
                                       0:D].rearrange(
                            "(p t) d -> p t d", p=128,
                            t=QROWS // 128),
                        rsb[:])
                nc.vector.tensor_add(
                    acc_t[:, q, :, :], acc_t[:, q, :, :], rsb[:])

            pending_rs = []     # [(layer, q, ready_ci)]
            pending_fin = []    # [(layer, q, ready_ci)]
            for layer in range(N_LAYERS):
                tbl = read_t[layer]
                blk_cnt = 0         # completed blocks in current chunk
                psum_t = None
                stage_t = None
                ti = 0
                for ci in range(n_calls):
                    idx_t = gpool.tile([128, NB * 8], dt.int16, tag="idx")
                    nc.sync.dma_start(
                        idx_t[:], idx_in[ci * 128:(ci + 1) * 128, :])
                    scol_t = gpool.tile([128, NB], dt.bfloat16, tag="scol")
                    nc.scalar.dma_start(
                        scol_t[:], scol_in[ci * 128:(ci + 1) * 128, :])
                    val_t = gpool.tile([128, NB], dt.bfloat16, tag="val")
                    nc.scalar.dma_start(
                        val_t[:], val_in[ci * 128:(ci + 1) * 128, :])
                    # ucode limit: 1024 idxs per dma_gather call; separate
                    # tiles so the 4 SWDGE queues actually overlap (slices of
                    # one tile get serialized by the dep tracker)
                    g_ts = []
                    for k in range(NB // 8):
                        g_k = gpool.tile([128, 8, ROWB], dt.bfloat16,
                                         tag=f"g{k}")
                        nc.gpsimd.dma_gather(
                            out_ap=g_k[:],
                            in_ap=tbl[:],
                            idxs_ap=idx_t[:, k * 64:(k + 1) * 64],
                            num_idxs=1024, num_idxs_reg=1024,
                            elem_size=ROWB, queue_num=qctr % 4)
                        qctr += 1
                        g_ts.append(g_k)
                    while pending_rs and pending_rs[0][2] <= ci:
                        lq = pending_rs.pop(0)
                        dispatch_rs(lq[0], lq[1])
                    while pending_fin and pending_fin[0][2] <= ci:
                        lq = pending_fin.pop(0)
                        finish_chunk(lq[0], lq[1])
                    s_t = spool.tile([128, NB, 128], dt.bfloat16, tag="s")
                    nc.vector.tensor_tensor(
                        s_t[:],
                        scol_t[:].unsqueeze(-1).broadcast_to([128, NB, 128]),
                        iota_t[:].unsqueeze(1).broadcast_to([128, NB, 128]),
                        mybir.AluOpType.is_equal)
                    gv_t = spool.tile([128, NB, D], dt.bfloat16, tag="gv")
                    for k in range(NB // 8):
                        nc.vector.tensor_tensor(
                            gv_t[:, k * 8:(k + 1) * 8, :],
                            g_ts[k][:, :, 0:D],
                            val_t[:, k * 8:(k + 1) * 8].unsqueeze(
                                -1).broadcast_to([128, 8, D]),
                            mybir.AluOpType.mult)

                    for t in range(NB):
                        b, first, last = tiles_meta[ti]
                        if first and blk_cnt % 4 == 0:
                            psum_t = ppool.tile([128, 4, D], dt.float32)
                        nc.tensor.matmul(
                            psum_t[:, blk_cnt % 4, :],
                            s_t[:, t, :], gv_t[:, t, :],
                            start=first, stop=last)
                        ti += 1
                        if not last:
                            continue
                        # block b complete
                        if blk_cnt % 8 == 0:
                            stage_t = spool.tile([128, 8, D], dt.bfloat16,
                                                 tag="stage")
                        if blk_cnt % 4 == 3:
                            nc.scalar.activation(
                                stage_t[:, (blk_cnt % 8) - 3:(blk_cnt % 8) + 1, :],
                                psum_t[:],
                                mybir.ActivationFunctionType.Copy)
                        if blk_cnt % 8 == 7:
                            q = _chunk_of_block(b)
                            r0 = _block_pprime_row(b) - q * CHROWS - 7 * 128
                            nc.sync.dma_start(
                                pp[layer][q][r0:r0 + 1024, :].rearrange(
                                    "(j p) d -> p j d", p=128, j=8),
                                stage_t[:])
                        blk_cnt += 1
                        if blk_cnt % BPC == 0:
                            # chunk q done; RS dispatched 2 calls later, the
                            # unshard/acc ~24 calls later (past RS completion)
                            q = blk_cnt // BPC - 1
                            pending_rs.append((layer, q, ci + 2))
                            pending_fin.append((layer, q, ci + 12))
                while pending_rs:
                    lq = pending_rs.pop(0)
                    dispatch_rs(lq[0], lq[1])
                while pending_fin:
                    lq = pending_fin.pop(0)
                    finish_chunk(lq[0], lq[1])

            # final: light = acc/4 -> AllGather -> windowed gathers -> dots
            nc.vector.tensor_scalar_mul(acc_t[:], acc_t[:], 0.25)
            nc.sync.dma_start(
                ag_in.ap().rearrange("(q p t) d -> p q t d",
                                     q=NCHUNK, p=128, t=QROWS // 128),
                acc_t[:])
            nc.gpsimd.collective_compute(
                "AllGather", mybir.AluOpType.bypass,
                replica_groups=[list(range(NC))],
                ins=[ag_in.ap()], outs=[A_tab.ap()])

            fu_t = pers.tile([128, FS // 16], dt.int16)
            nc.sync.dma_start(fu_t[:], fu_in[:])
            fi_t = pers.tile([128, FS // 16], dt.int16)
            nc.sync.dma_start(fi_t[:], fi_in[:])
            ug_t = pers.tile([128, FT, D], dt.float32)
            ig_t = pers.tile([128, FT, D], dt.float32)
            for subs, idxt, outt in ((fin_subs[0], fu_t, ug_t),
                                     (fin_subs[1], fi_t, ig_t)):
                for w, lo, hi in subs:
                    for a in range(lo, hi, 1024):
                        n = min(1024, hi - a)
                        nc.gpsimd.dma_gather(
                            out_ap=outt[:, a // 128:(a + n) // 128, :],
                            in_ap=A_tab[w * WIN:(w + 1) * WIN, :],
                            idxs_ap=idxt[:, a // 16:(a + n) // 16],
                            num_idxs=n, num_idxs_reg=n,
                            elem_size=D, queue_num=qctr % 4)
                        qctr += 1
            prod_t = pers.tile([128, FT, D], dt.float32)
            nc.vector.tensor_mul(prod_t[:], ug_t[:], ig_t[:])
            gam_t = pers.tile([128, FT], dt.float32)
            nc.vector.tensor_reduce(
                gam_t[:].unsqueeze(-1), prod_t[:],
                axis=mybir.AxisListType.X, op=mybir.AluOpType.add)
            nc.sync.dma_start(gamma_out[:], gam_t[:])

    nc.compile()
    return nc


def kernel(**inputs):
    from concourse import bass_utils

    users = np.asarray(inputs["users"])
    items = np.asarray(inputs["items"])
    edge_src = np.asarray(inputs["edge_src"])
    edge_dst = np.asarray(inputs["edge_dst"])
    edge_val = np.asarray(inputs["edge_val"], dtype=np.float32)
    user_emb = np.asarray(inputs["user_emb"], dtype=np.float32)
    item_emb = np.asarray(inputs["item_emb"], dtype=np.float32)

    sched, per_core, S = _prep_edges(edge_src, edge_dst, edge_val)
    T_tiles = S // 128
    plans, combos_sizes, FS = _prep_final(users, items)
    fin_subs = (plans[0]["usub"], plans[0]["isub"])

    nc = _build_program(sched, T_tiles, fin_subs, FS)

    x0 = np.zeros((N_PAD, D), dtype=np.float32)
    x0[:N_USERS] = user_emb
    x0[N_USERS:N] = item_emb
    iota_img = np.tile(np.arange(128, dtype=np.float32).astype(BF)[None, :],
                       (128, 1)).copy()

    in_maps = []
    for c in range(NC):
        idx16, scol, val = per_core[c]
        t0 = np.zeros((WIN, ROWB), dtype=BF)
        t0[:, :D] = x0[c * WIN:(c + 1) * WIN].astype(BF)
        pl = plans[c]
        in_maps.append({
            "t0": t0,
            "x0": x0[c * WIN:(c + 1) * WIN].copy(),
            "idx": _wrap_idx_ci(idx16),
            "scol": _wrap_slots_ci(scol.astype(BF)),
            "val": _wrap_slots_ci(val.astype(BF)),
            "iota": iota_img,
            "fuidx": _wrap_idx(pl["uidx"]),
            "fiidx": _wrap_idx(pl["iidx"]),
        })

    res = bass_utils.run_bass_kernel_spmd(
        nc, in_maps, core_ids=list(range(NC)),
        trace=bool(os.environ.get("KERNEL_TRACE")))
    global LAST_RESULTS
    LAST_RESULTS = res

    gamma = np.zeros(users.shape[0], dtype=np.float32)
    for c in range(NC):
        img = res.results[c]["gamma"]          # [128, FT]
        flat = img.T.reshape(-1)               # slot s = 128*t + p
        pl = plans[c]
        sel = pl["slots_pair"] >= 0
        gamma[pl["slots_pair"][sel]] = flat[sel]
    return gamma



# revision 30
# speedup vs baseline: 1.0096x; 1.0048x over previous
"""LightGCN on 8 Trainium2 NeuronCores (Bass/Tile).

Scheme (src-sharded, ReduceScatter):
- Node table padded to N_PAD = 8*18816 rows, stored per core as a bf16
  [WIN, 128] window table (cols 0:64 = features, 64:128 = don't-care pad so
  rows are 256B for dma_gather).
- Edge (src, dst, val) is processed by core src // WIN. Per core, edges are
  bucketed into 1176 dst-blocks of 128 dsts, laid out in 128-slot tiles;
  per-block tile counts are equalized across cores so one program serves all.
- Per tile: dma_gather x[src] rows (bf16), build the one-hot
  S[slot, dstcol] = (scol == iota) with one DVE op, fold val into the
  gathered rows (gv = g * val, DVE), TensorEngine matmul S^T @ gv
  accumulates the block's [128, 64] partial sums in PSUM.
- Gathers round-robin over 4 SWDGE queues (num_swdge_queues=4) into
  independent tiles so the 4 gpsimd core pairs generate descriptors in
  parallel (the single-queue ucode is the kernel's bottleneck at ~9ns/row).
- Partials are cast to bf16 and go to a permuted DRAM table P' so a chunked
  ReduceScatter (7 chunks of 21*128 rows per window) hands each core exactly
  its window's new embeddings. The RS dispatch is emitted 2 gather-calls
  after a chunk completes and the unshard/acc/table-write 12 calls after, so
  their semaphore waits don't stall the in-order gpsimd/vector/scalar/sync
  queues that the gather pipeline runs on.
- idx/scol/val are laid out per-call-contiguous in DRAM ([n_ci*128, .])
  so each call's metadata load is a few large linear DMA packets; scol/val
  load via the scalar-engine HWDGE queue to stay off the sync queue.
- acc accumulates per-shard in SBUF f32; final AllGather + windowed gathers
  of user/item rows + dot products produce gamma.
"""
import os
import numpy as np
import ml_dtypes

BF = ml_dtypes.bfloat16
LAST_RESULTS = None  # BassKernelResults of the most recent run (for test.py)

N_USERS = 100000
N_ITEMS = 50000
N = N_USERS + N_ITEMS
D = 64
NC = 8
WIN = 18816             # 147 * 128 nodes per core window
N_PAD = NC * WIN        # 150528
NBLOCKS = N_PAD // 128  # 1176
NCHUNK = 7
QROWS = WIN // NCHUNK   # 2688 = 21*128
CHROWS = NC * QROWS     # 50176 rows per P' chunk
BPC = NBLOCKS // NCHUNK # 392 blocks per chunk
ROWB = 128              # table row elems (256B)
NB = 48                 # tiles per dma_gather call
NBV = 24                # tiles of the val-multiply done on DVE (rest scalar)
N_LAYERS = 3


def _chunk_of_block(b):
    return ((b * 128) % WIN) // QROWS


def _block_pprime_row(b):
    n = b * 128
    w, r = divmod(n, WIN)
    q, k = divmod(r, QROWS)
    return (q * NC + w) * QROWS + k


def _prep_edges(edge_src, edge_dst, edge_val):
    src = edge_src.astype(np.int64)
    dst = edge_dst.astype(np.int64)
    val = edge_val.astype(np.float32)
    core = src // WIN
    blk = dst // 128

    counts = np.zeros((NC, NBLOCKS), dtype=np.int64)
    np.add.at(counts, (core, blk), 1)
    ntiles = np.maximum(1, (counts.max(axis=0) + 127) // 128)

    order = sorted(range(NBLOCKS), key=lambda b: (_chunk_of_block(b), b))
    # pad total tiles to a multiple of NB (extra zero tiles on the last block)
    tot = int(ntiles[order].sum())
    pad_tiles = (-tot) % NB
    ntiles[order[-1]] += pad_tiles
    sched = [(b, int(ntiles[b])) for b in order]

    slot_of_block = {}
    off = 0
    for b, t in sched:
        slot_of_block[b] = off
        off += t * 128
    S = off

    ordk = np.lexsort((dst, blk, core))
    src_s, dst_s, val_s, core_s, blk_s = (
        src[ordk], dst[ordk], val[ordk], core[ordk], blk[ordk])
    key = core_s * NBLOCKS + blk_s
    per_core = []
    for c in range(NC):
        idx16 = np.zeros(S, dtype=np.int16)
        scol = np.zeros(S, dtype=np.float32)
        v = np.zeros(S, dtype=np.float32)
        lo = np.searchsorted(key, c * NBLOCKS)
        hi = np.searchsorted(key, (c + 1) * NBLOCKS)
        bsrc, bdst, bval, bblk = (
            src_s[lo:hi], dst_s[lo:hi], val_s[lo:hi], blk_s[lo:hi])
        bounds = np.searchsorted(bblk, np.arange(NBLOCKS + 1))
        for b in range(NBLOCKS):
            a, e = bounds[b], bounds[b + 1]
            if e == a:
                continue
            o = slot_of_block[b]
            n = e - a
            idx16[o:o + n] = (bsrc[a:e] - c * WIN).astype(np.int16)
            scol[o:o + n] = (bdst[a:e] - b * 128).astype(np.float32)
            v[o:o + n] = bval[a:e]
        per_core.append((idx16, scol, v))
    return sched, per_core, S


def _wrap_idx(idx_flat):
    S = idx_flat.shape[0]
    w = idx_flat.reshape(S // 16, 16).T
    return np.tile(w, (8, 1)).copy()


def _wrap_idx_ci(idx_flat):
    # [n_ci*128, NB*8]: per-ci contiguous blocks of the wrapped image
    w = _wrap_idx(idx_flat)              # [128, T*8]
    T8 = w.shape[1]
    n_ci = T8 // (NB * 8)
    return np.concatenate(
        [w[:, i * NB * 8:(i + 1) * NB * 8] for i in range(n_ci)],
        axis=0).copy()


def _wrap_slots_ci(a_flat):
    w = _wrap_slots(a_flat)              # [128, T]
    T = w.shape[1]
    n_ci = T // NB
    return np.concatenate(
        [w[:, i * NB:(i + 1) * NB] for i in range(n_ci)], axis=0).copy()


def _wrap_slots(a_flat):
    S = a_flat.shape[0]
    return a_flat.reshape(S // 128, 128).T.copy()


def _prep_final(users, items):
    B = users.shape[0]
    pcn = B // NC
    u = users.astype(np.int64)
    it = items.astype(np.int64) + N_USERS
    uw = u // WIN
    iw = it // WIN
    combos = sorted({(int(a), int(b)) for a, b in zip(uw, iw)})
    gmax = {}
    for c in range(NC):
        lo, hi = c * pcn, (c + 1) * pcn
        for cu, ci in combos:
            m = int(((uw[lo:hi] == cu) & (iw[lo:hi] == ci)).sum())
            gmax[(cu, ci)] = max(gmax.get((cu, ci), 0), m)
    combos_sizes = [(k, (gmax[k] + 127) // 128 * 128) for k in combos]
    pad_total = sum(s for _, s in combos_sizes)
    plans = []
    for c in range(NC):
        lo, hi = c * pcn, (c + 1) * pcn
        slots_pair = np.full(pad_total, -1, dtype=np.int64)
        uidx = np.zeros(pad_total, dtype=np.int16)
        iidx = np.zeros(pad_total, dtype=np.int16)
        usub, isub = [], []
        off = 0
        for (cu, ci), size in combos_sizes:
            sel = np.nonzero((uw[lo:hi] == cu) & (iw[lo:hi] == ci))[0]
            n = sel.shape[0]
            slots_pair[off:off + n] = lo + sel
            uidx[off:off + n] = (u[lo + sel] - cu * WIN).astype(np.int16)
            iidx[off:off + n] = (it[lo + sel] - ci * WIN).astype(np.int16)
            usub.append((cu, off, off + size))
            isub.append((ci, off, off + size))
            off += size
        plans.append({"slots_pair": slots_pair, "uidx": uidx, "iidx": iidx,
                      "usub": usub, "isub": isub, "ntot": pad_total})
    return plans, combos_sizes, pad_total


def _build_program(sched, T_tiles, fin_subs, FS):
    """Build + compile the shared 8-core program.

    sched: [(block, ntiles)] chunk-major. T_tiles = total tiles per layer.
    fin_subs: (usub, isub) window sub-ranges (identical across cores).
    FS: final slot count (multiple of 128).
    """
    import concourse.bacc as bacc
    import concourse.tile as tile
    from concourse import mybir

    FT = FS // 128
    nc = bacc.Bacc("TRN2", target_bir_lowering=False, debug=False,
                   num_devices=NC, num_swdge_queues=4)
    dt = mybir.dt

    t0_in = nc.dram_tensor("t0", [WIN, ROWB], dt.bfloat16, kind="ExternalInput")
    x0_in = nc.dram_tensor("x0", [WIN, D], dt.float32, kind="ExternalInput")
    n_ci = T_tiles // NB
    idx_in = nc.dram_tensor("idx", [n_ci * 128, NB * 8], dt.int16,
                            kind="ExternalInput")
    scol_in = nc.dram_tensor("scol", [n_ci * 128, NB], dt.bfloat16,
                             kind="ExternalInput")
    val_in = nc.dram_tensor("val", [n_ci * 128, NB], dt.bfloat16,
                            kind="ExternalInput")
    iota_in = nc.dram_tensor("iota", [128, 128], dt.bfloat16,
                             kind="ExternalInput")
    fu_in = nc.dram_tensor("fuidx", [128, FS // 16], dt.int16,
                           kind="ExternalInput")
    fi_in = nc.dram_tensor("fiidx", [128, FS // 16], dt.int16,
                           kind="ExternalInput")
    gamma_out = nc.dram_tensor("gamma", [128, FT], dt.float32,
                               kind="ExternalOutput")

    tp = [nc.dram_tensor(f"tbl{i}", [WIN, ROWB], dt.bfloat16, kind="Internal")
          for i in range(2)]
    read_t = [t0_in, tp[0], tp[1]]
    write_t = [tp[0], tp[1], None]
    A_tab = nc.dram_tensor("atab", [N_PAD, D], dt.float32, kind="Internal")
    ag_in = nc.dram_tensor("agin", [WIN, D], dt.float32, kind="Internal")
    pp = [[nc.dram_tensor(f"pp_l{l}_q{q}", [CHROWS, D], dt.bfloat16,
                          kind="Internal") for q in range(NCHUNK)]
          for l in range(N_LAYERS)]
    rs = [[nc.dram_tensor(f"rs_l{l}_q{q}", [QROWS, D], dt.bfloat16,
                          kind="Internal")
           for q in range(NCHUNK)] for l in range(N_LAYERS)]

    n_calls = T_tiles // NB
    # flat per-tile (block, is_first, is_last) in sched order
    tiles_meta = []
    for b, nt in sched:
        for k in range(nt):
            tiles_meta.append((b, k == 0, k == nt - 1))
    assert len(tiles_meta) == T_tiles

    with tile.TileContext(nc) as tc:
        with tc.tile_pool(name="persist", bufs=1) as pers, \
             tc.tile_pool(name="gbuf", bufs=5) as gpool, \
             tc.tile_pool(name="sbuf2", bufs=2) as spool, \
             tc.tile_pool(name="rbuf", bufs=2) as rpool, \
             tc.tile_pool(name="psum", bufs=4, space="PSUM") as ppool:

            iota_t = pers.tile([128, 128], dt.bfloat16)
            nc.sync.dma_start(iota_t[:], iota_in[:])
            acc_t = pers.tile([128, NCHUNK, QROWS // 128, D], dt.float32)
            nc.sync.dma_start(
                acc_t[:],
                x0_in.ap().rearrange("(q p t) d -> p q t d",
                                     q=NCHUNK, p=128, t=QROWS // 128))

            qctr = 0            # SWDGE queue round-robin counter

            def dispatch_rs(layer, q):
                nc.gpsimd.collective_compute(
                    "ReduceScatter", mybir.AluOpType.add,
                    replica_groups=[list(range(NC))],
                    ins=[pp[layer][q].ap()],
                    outs=[rs[layer][q].ap()])

            def finish_chunk(layer, q):
                """Unshard + acc for a chunk whose RS has (long since)
                completed. Emitted ~24 calls late so the sem waits on the
                in-order vector/scalar/sync queues are already satisfied."""
                rsb = rpool.tile([128, QROWS // 128, D],
                                 dt.bfloat16, tag="rsb")
                nc.sync.dma_start(
                    rsb[:],
                    rs[layer][q].ap().rearrange(
                        "(p t) d -> p t d", p=128,
                        t=QROWS // 128))
                if write_t[layer] is not None:
                    nc.sync.dma_start(
                        write_t[layer][q * QROWS:(q + 1) * QROWS,
                                       0:D].rearrange(
                            "(p t) d -> p t d", p=128,
                            t=QROWS // 128),
                        rsb[:])
                nc.vector.tensor_add(
                    acc_t[:, q, :, :], acc_t[:, q, :, :], rsb[:])

            pending_rs = []     # [(layer, q, ready_ci)]
            pending_fin = []    # [(layer, q, ready_ci)]
            for layer in range(N_LAYERS):
                tbl = read_t[layer]
                blk_cnt = 0         # completed blocks in current chunk
                psum_t = None
                stage_t = None
                ti = 0
                for ci in range(n_calls):
                    idx_t = gpool.tile([128, NB * 8], dt.int16, tag="idx")
                    nc.sync.dma_start(
                        idx_t[:], idx_in[ci * 128:(ci + 1) * 128, :])
                    scol_t = gpool.tile([128, NB], dt.bfloat16, tag="scol")
                    nc.scalar.dma_start(
                        scol_t[:], scol_in[ci * 128:(ci + 1) * 128, :])
                    val_t = gpool.tile([128, NB], dt.bfloat16, tag="val")
                    nc.scalar.dma_start(
                        val_t[:], val_in[ci * 128:(ci + 1) * 128, :])
                    # ucode limit: 1024 idxs per dma_gather call; separate
                    # tiles so the 4 SWDGE queues actually overlap (slices of
                    # one tile get serialized by the dep tracker)
                    g_ts = []
                    for k in range(NB // 8):
                        g_k = gpool.tile([128, 8, ROWB], dt.bfloat16,
                                         tag=f"g{k}")
                        nc.gpsimd.dma_gather(
                            out_ap=g_k[:],
                            in_ap=tbl[:],
                            idxs_ap=idx_t[:, k * 64:(k + 1) * 64],
                            num_idxs=1024, num_idxs_reg=1024,
                            elem_size=ROWB, queue_num=qctr % 4)
                        qctr += 1
                        g_ts.append(g_k)
                    while pending_rs and pending_rs[0][2] <= ci:
                        lq = pending_rs.pop(0)
                        dispatch_rs(lq[0], lq[1])
                    while pending_fin and pending_fin[0][2] <= ci:
                        lq = pending_fin.pop(0)
                        finish_chunk(lq[0], lq[1])
                    s_t = spool.tile([128, NB, 128], dt.bfloat16, tag="s")
                    nc.vector.tensor_tensor(
                        s_t[:],
                        scol_t[:].unsqueeze(-1).broadcast_to([128, NB, 128]),
                        iota_t[:].unsqueeze(1).broadcast_to([128, NB, 128]),
                        mybir.AluOpType.is_equal)
                    gv_t = spool.tile([128, NB, D], dt.bfloat16, tag="gv")
                    for k in range(NB // 8):
                        nc.vector.tensor_tensor(
                            gv_t[:, k * 8:(k + 1) * 8, :],
                            g_ts[k][:, :, 0:D],
                            val_t[:, k * 8:(k + 1) * 8].unsqueeze(
                                -1).broadcast_to([128, 8, D]),
                            mybir.AluOpType.mult)

                    for t in range(NB):
                        b, first, last = tiles_meta[ti]
                        if first and blk_cnt % 4 == 0:
                            psum_t = ppool.tile([128, 4, D], dt.float32)
                        nc.tensor.matmul(
                            psum_t[:, blk_cnt % 4, :],
                            s_t[:, t, :], gv_t[:, t, :],
                            start=first, stop=last)
                        ti += 1
                        if not last:
                            continue
                        # block b complete
                        if blk_cnt % 8 == 0:
                            stage_t = spool.tile([128, 8, D], dt.bfloat16,
                                                 tag="stage")
                        if blk_cnt % 4 == 3:
                            nc.scalar.activation(
                                stage_t[:, (blk_cnt % 8) - 3:(blk_cnt % 8) + 1, :],
                                psum_t[:],
                                mybir.ActivationFunctionType.Copy)
                        if blk_cnt % 8 == 7:
                            q = _chunk_of_block(b)
                            r0 = _block_pprime_row(b) - q * CHROWS - 7 * 128
                            nc.sync.dma_start(
                                pp[layer][q][r0:r0 + 1024, :].rearrange(
                                    "(j p) d -> p j d", p=128, j=8),
                                stage_t[:])
                        blk_cnt += 1
                        if blk_cnt % BPC == 0:
                            # chunk q done; RS dispatched 2 calls later, the
                            # unshard/acc ~24 calls later (past RS completion)
                            q = blk_cnt // BPC - 1
                            pending_rs.append((layer, q, ci + 2))
                            pending_fin.append((layer, q, ci + 12))
                while pending_rs:
                    lq = pending_rs.pop(0)
                    dispatch_rs(lq[0], lq[1])
                while pending_fin:
                    lq = pending_fin.pop(0)
                    finish_chunk(lq[0], lq[1])

            # final: light = acc/4 -> AllGather -> windowed gathers -> dots
            nc.vector.tensor_scalar_mul(acc_t[:], acc_t[:], 0.25)
            nc.sync.dma_start(
                ag_in.ap().rearrange("(q p t) d -> p q t d",
                                     q=NCHUNK, p=128, t=QROWS // 128),
                acc_t[:])
            nc.gpsimd.collective_compute(
                "AllGather", mybir.AluOpType.bypass,
                replica_groups=[list(range(NC))],
                ins=[ag_in.ap()], outs=[A_tab.ap()])

            fu_t = pers.tile([128, FS // 16], dt.int16)
            nc.sync.dma_start(fu_t[:], fu_in[:])
            fi_t = pers.tile([128, FS // 16], dt.int16)
            nc.sync.dma_start(fi_t[:], fi_in[:])
            ug_t = pers.tile([128, FT, D], dt.float32)
            ig_t = pers.tile([128, FT, D], dt.float32)
            for subs, idxt, outt in ((fin_subs[0], fu_t, ug_t),
                                     (fin_subs[1], fi_t, ig_t)):
                for w, lo, hi in subs:
                    for a in range(lo, hi, 1024):
                        n = min(1024, hi - a)
                        nc.gpsimd.dma_gather(
                            out_ap=outt[:, a // 128:(a + n) // 128, :],
                            in_ap=A_tab[w * WIN:(w + 1) * WIN, :],
                            idxs_ap=idxt[:, a // 16:(a + n) // 16],
                            num_idxs=n, num_idxs_reg=n,
                            elem_size=D, queue_num=qctr % 4)
                        qctr += 1
            prod_t = pers.tile([128, FT, D], dt.float32)
            nc.vector.tensor_mul(prod_t[:], ug_t[:], ig_t[:])
            gam_t = pers.tile([128, FT], dt.float32)
            nc.vector.tensor_reduce(
                gam_t[:].unsqueeze(-1), prod_t[:],
                axis=mybir.AxisListType.X, op=mybir.AluOpType.add)
            nc.sync.dma_start(gamma_out[:], gam_t[:])

    nc.compile()
    return nc


def kernel(**inputs):
    from concourse import bass_utils

    users = np.asarray(inputs["users"])
    items = np.asarray(inputs["items"])
    edge_src = np.asarray(inputs["edge_src"])
    edge_dst = np.asarray(inputs["edge_dst"])
    edge_val = np.asarray(inputs["edge_val"], dtype=np.float32)
    user_emb = np.asarray(inputs["user_emb"], dtype=np.float32)
    item_emb = np.asarray(inputs["item_emb"], dtype=np.float32)

    sched, per_core, S = _prep_edges(edge_src, edge_dst, edge_val)
    T_tiles = S // 128
    plans, combos_sizes, FS = _prep_final(users, items)
    fin_subs = (plans[0]["usub"], plans[0]["isub"])

    nc = _build_program(sched, T_tiles, fin_subs, FS)

    x0 = np.zeros((N_PAD, D), dtype=np.float32)
    x0[:N_USERS] = user_emb
    x0[N_USERS:N] = item_emb
    iota_img = np.tile(np.arange(128, dtype=np.float32).astype(BF)[None, :],
                       (128, 1)).copy()

    in_maps = []
    for c in range(NC):
        idx16, scol, val = per_core[c]
        t0 = np.zeros((WIN, ROWB), dtype=BF)
        t0[:, :D] = x0[c * WIN:(c + 1) * WIN].astype(BF)
        pl = plans[c]
        in_maps.append({
            "t0": t0,
            "x0": x0[c * WIN:(c + 1) * WIN].copy(),
            "idx": _wrap_idx_ci(idx16),
            "scol": _wrap_slots_ci(scol.astype(BF)),
            "val": _wrap_slots_ci(val.astype(BF)),
            "iota": iota_img,
            "fuidx": _wrap_idx(pl["uidx"]),
            "fiidx": _wrap_idx(pl["iidx"]),
        })

    res = bass_utils.run_bass_kernel_spmd(
        nc, in_maps, core_ids=list(range(NC)),
        trace=bool(os.environ.get("KERNEL_TRACE")))
    global LAST_RESULTS
    LAST_RESULTS = res

    gamma = np.zeros(users.shape[0], dtype=np.float32)
    for c in range(NC):
        img = res.results[c]["gamma"]          # [128, FT]
        flat = img.T.reshape(-1)               # slot s = 128*t + p
        pl = plans[c]
        sel = pl["slots_pair"] >= 0
        gamma[pl["slots_pair"][sel]] = flat[sel]
    return gamma

